# revision 38
# baseline (speedup 1.0000x reference)
"""DiffAttn TRN2 kernel: 8-core SPMD (batch x query-half sharding).

Algebraic restructuring: softmax_a's logits are x_q (Wq_a Wk_a^T) x^T, so
the host precomputes M_a = Wq_a Wk_a^T (weight preprocessing) and the
device computes u_a = M_a^T x_q^T once per core (q-half specific, so
nothing is duplicated across the pair), with raw x^T serving as the
keys. The output side is likewise reassociated: out^T = Wv^T (x^T
diff^T), which removes the V projection over the full sequence. Per
core: ~1240 N=512 matmuls vs 1856 for the q/k/v-projection formulation.

All tensors are fp16 and SBUF-resident (no DRAM staging, ~16MB HBM
traffic/core). Layouts: scores come out [s-part, q-free] so diff feeds
the wT = x^T diff^T contraction directly; out^T lands [d-part, q-free]
and the RMS d-reduction is a ones-matmul on the squared tensor. The
per-q (column) broadcasts of 1/den and the RMS scale are rank-1 PE
matmuls (ones-row outer product, with lambda / (1-lambda_init) folded
into the stationary row) — keeps GpSimd idle and the Scalar FIFO free
for the exp pipeline.
"""

import sys

for _p in ("/opt/trn_rl_repo", "/root/.axon_site/_ro/trn_rl_repo"):
    if _p not in sys.path:
        sys.path.append(_p)

import numpy as np

import concourse.bass as bass
import concourse.mybir as mybir
from concourse import bacc
from concourse.bass_utils import run_bass_kernel_spmd
from concourse.tile import TileContext

F32 = mybir.dt.float32
F32R = mybir.dt.float32r
F16 = mybir.dt.float16
AF = mybir.ActivationFunctionType

D = 1024          # embed dim
S = 2048          # sequence length
B = 4             # batch
NCORES = 8
QH = 1024         # query rows per core (half a sequence)
QB = 512          # query block (matmul moving dim)
NQB = QH // QB    # 2
NDT = D // 128    # 8 d tiles
NST = S // 128    # 16 s tiles
LAMBDA_INIT = 0.8
EPS = 1e-5
SCALE = float(D) ** -0.25

_CACHE = {}


def _build_nc():
    nc = bacc.Bacc("TRN2", target_bir_lowering=False, debug=False,
                   num_devices=NCORES)

    xT = nc.declare_dram_parameter("xT", [D, S], F16, isOutput=False)
    xsd = nc.declare_dram_parameter("xsd", [S, D], F16, isOutput=False)
    mm = nc.declare_dram_parameter("mm", [D, 2 * D], F16, isOutput=False)
    wv = nc.declare_dram_parameter("wv", [D, D], F16, isOutput=False)
    lamv_d = nc.declare_dram_parameter("lamv", [1, 1], F32, isOutput=False)
    out = nc.declare_dram_parameter("out", [D, QH], F16, isOutput=True)
    rrow = nc.declare_dram_parameter("rrow", [1, QH], F32, isOutput=True)

    xT_v = xT.ap().rearrange("(dt p) s -> p dt s", p=128)
    xsd_v = xsd.ap().rearrange("(st p) d -> st p d", p=128)
    mm_v = mm.ap().rearrange("(dt p) e -> p dt e", p=128)
    wv_v = wv.ap().rearrange("(dt p) e -> p dt e", p=128)
    out_v = out.ap().rearrange("(t p) q -> t p q", p=128)     # [8,128,QH]

    with TileContext(nc) as tc:
        singles_cm = tc.tile_pool(name="singles", bufs=1)
        singles = singles_cm.__enter__()

        lamv = singles.tile([1, 1], F32)
        nc.sync.dma_start(out=lamv, in_=lamv_d.ap())
        ones_col = singles.tile([128, 1], F16)
        nc.vector.memset(ones_col, 1.0)
        ones_row = singles.tile([1, 128], F16)
        nc.vector.memset(ones_row, 1.0)
        lam_row_f = singles.tile([1, 128], F32)
        nc.vector.memset(lam_row_f, 1.0)
        nc.vector.tensor_scalar_mul(lam_row_f, lam_row_f, lamv)
        lam_row = singles.tile([1, 128], F16)
        nc.scalar.copy(lam_row, lam_row_f)
        eps1 = singles.tile([1, 1], F32)
        nc.vector.memset(eps1, EPS)
        negc = singles.tile([128, 1], F32)
        nc.vector.memset(negc, -3.0)

        # ---- resident tensors --------------------------------------------
        pres_cm = tc.tile_pool(name="pres", bufs=1)
        pres = pres_cm.__enter__()
        xT_sb = pres.tile([128, NDT, S], F16)
        xsd_sb = pres.tile([128, NST, D], F16)
        wv_sb = pres.tile([128, NDT, D], F16)
        u_sb = pres.tile([128, 2 * NDT, QH], F16)

        # u-proj inputs, freed after phase U
        pu_in_cm = tc.tile_pool(name="puin", bufs=1, side="right")
        pu_in = pu_in_cm.__enter__()
        mm_sb = pu_in.tile([128, NDT, 2 * D], F16)

        # DMA issue order: the first u accumulation chain needs
        # mm[:, din, 0:128] + xq[:, din, 0:512] for every din, so those
        # thin slices go first; bulk follows; keys/values last.
        for din in range(NDT):
            nc.sync.dma_start(out=mm_sb[:, din, 0:128],
                              in_=mm_v[:, din, 0:128])
            nc.sync.dma_start(out=xT_sb[:, din, 0:QB],
                              in_=xT_v[:, din, 0:QB])
        for lo, hi in ((128, 384), (384, 640), (640, 896), (896, D)):
            for din in range(NDT):
                nc.sync.dma_start(out=mm_sb[:, din, lo:hi],
                                  in_=mm_v[:, din, lo:hi])
        for din in range(NDT):
            nc.sync.dma_start(out=xT_sb[:, din, QB:QH],
                              in_=xT_v[:, din, QB:QH])
        for lo, hi in ((D, D + 256), (D + 256, D + 512),
                       (D + 512, D + 768), (D + 768, 2 * D)):
            for din in range(NDT):
                nc.sync.dma_start(out=mm_sb[:, din, lo:hi],
                                  in_=mm_v[:, din, lo:hi])
        for dt in range(NDT):
            nc.sync.dma_start(out=xT_sb[:, dt, QH:S],
                              in_=xT_v[:, dt, QH:S])
        for st in range(NST):
            nc.sync.dma_start(out=xsd_sb[:, st, :], in_=xsd_v[st])
        for dt in range(NDT):
            nc.sync.dma_start(out=wv_sb[:, dt, :], in_=wv_v[:, dt, :])

        # ---- phase U: u_a = M_a^T x_q^T  ---------------------------------
        psu_cm = tc.tile_pool(name="psu", bufs=2, space="PSUM")
        psu = psu_cm.__enter__()
        # PE warmup during the input-DMA window: flips the HAM clock gate
        # to 8/8 before real matmuls arrive (no consumers, zero operands)
        wzero = singles.tile([128, QB], F16)
        nc.vector.memset(wzero, 0.0)
        pwm = psu.tile([1, QB], F32, tag="warm", name="pwm", bufs=1)
        for i in range(8):
            nc.tensor.matmul(pwm, lhsT=ones_col, rhs=wzero,
                             start=True, stop=True)
        nwarm = 0
        for a in range(2):
            for qc in range(NQB):
                for ot in range(NDT):
                    pu = psu.tile([128, QB], F32, name="pu")
                    for din in range(NDT):
                        nc.tensor.matmul(
                            pu,
                            lhsT=mm_sb[:, din,
                                       a * D + ot * 128:a * D + (ot + 1) * 128],
                            rhs=xT_sb[:, din, qc * QB:(qc + 1) * QB],
                            start=(din == 0), stop=(din == NDT - 1))
                    nc.vector.tensor_copy(
                        u_sb[:, a * NDT + ot, qc * QB:(qc + 1) * QB], pu)
                    if nwarm < 24:
                        # keep the PE activity monitor busy through the
                        # DMA-paced ramp so HAM reaches 8/8 early
                        nc.tensor.matmul(pwm, lhsT=ones_col, rhs=wzero,
                                         start=True, stop=True)
                        nwarm += 1
        psu_cm.__exit__(None, None, None)
        pu_in_cm.__exit__(None, None, None)

        # ---- attention ---------------------------------------------------
        with tc.tile_pool(name="eblk", bufs=1) as eblk, \
             tc.tile_pool(name="wtp", bufs=2) as wtp, \
             tc.tile_pool(name="otp", bufs=2) as otp, \
             tc.tile_pool(name="sqp", bufs=1) as sqp, \
             tc.tile_pool(name="work", bufs=2) as work, \
             tc.tile_pool(name="pssc", bufs=3, space="PSUM") as pssc, \
             tc.tile_pool(name="psdb", bufs=2, space="PSUM") as psdb, \
             tc.tile_pool(name="pswo", bufs=3, space="PSUM") as pswo:
            for bi in range(NQB):
                qs = bi * QB
                eT = {}
                bb = {}
                for a in (0, 1):
                    eT[a] = eblk.tile([128, NST, QB], F16,
                                      tag=f"e{a}", name=f"eT{a}")
                    pden = psdb.tile([128, QB], F32, tag="den", name="pden",
                                     bufs=1)
                    for st in range(NST):
                        psc = pssc.tile([128, QB], F32, tag="sc", name="psc")
                        for dt in range(NDT):
                            nc.tensor.matmul(
                                psc,
                                lhsT=xT_sb[:, dt, st * 128:(st + 1) * 128],
                                rhs=u_sb[:, a * NDT + dt, qs:qs + QB],
                                start=(dt == 0), stop=(dt == NDT - 1))
                        nc.scalar.activation(eT[a][:, st, :], psc, AF.Exp,
                                             scale=SCALE, bias=negc)
                    # 2 concurrent col-group chains (k-major issue order)
                    for k in range(8):
                        for g in range(2):
                            st = g * 8 + k
                            nc.tensor.matmul(
                                pden[32 * g:32 * g + 1, :], lhsT=ones_col,
                                rhs=eT[a][:, st, :],
                                start=(k == 0), stop=(k == 7),
                                tile_position=(0, 32 * g))
                    d01 = work.tile([1, QB], F32, tag="d01", name="d01",
                                    bufs=1)
                    nc.vector.tensor_copy(d01, pden[0:1, :])
                    den = work.tile([1, QB], F32, tag="dsum", name="den",
                                    bufs=1)
                    nc.vector.tensor_add(den, d01, pden[32:33, :])
                    rden = work.tile([1, QB], F32, tag="rden", name="rden",
                                     bufs=2)
                    nc.vector.reciprocal_approx_fast(rden, den)
                    rden_h = work.tile([1, QB], F16, tag="rdh", name="rden_h",
                                       bufs=2)
                    nc.vector.tensor_copy(rden_h, rden)
                    # column broadcast on PE: bb = row^T (x) rden, with
                    # lambda folded into the a=1 stationary row
                    bb_ps = psdb.tile([128, QB], F32, tag="den", name="bb_ps", bufs=1)
                    nc.tensor.matmul(bb_ps,
                                     lhsT=(ones_row if a == 0 else lam_row),
                                     rhs=rden_h, start=True, stop=True)
                    bb[a] = work.tile([128, QB], F16, tag=f"b{a}",
                                      name=f"bb{a}", bufs=1)
                    nc.vector.tensor_copy(bb[a], bb_ps)
                    if a == 0:
                        # runs on DVE while PE does attn-1 scores
                        for st in range(NST):
                            nc.vector.tensor_mul(eT[0][:, st, :],
                                                 eT[0][:, st, :], bb[0])
                    else:
                        for st in range(NST):
                            nc.vector.tensor_mul(eT[1][:, st, :],
                                                 eT[1][:, st, :], bb[1])
                            nc.vector.tensor_sub(eT[0][:, st, :],
                                                 eT[0][:, st, :],
                                                 eT[1][:, st, :])
                # preload the Sqrt activation table while PE runs wT/outT
                # so the bi-final rms doesn't pay the table switch in-chain
                sqwarm = work.tile([1, 1], F32, tag="sqw", name="sqw", bufs=1)
                nc.scalar.activation(sqwarm, eps1, AF.Sqrt)
                # wT = x^T diff^T  [d-part, q]
                wt_sb = wtp.tile([128, NDT, QB], F16, tag="wt", name="wt")
                for dt in range(NDT):
                    pw = pswo.tile([128, QB], F32, tag="w", name="pw", bufs=2)
                    for st in range(NST):
                        nc.tensor.matmul(
                            pw,
                            lhsT=xsd_sb[:, st, dt * 128:(dt + 1) * 128],
                            rhs=eT[0][:, st, :],
                            start=(st == 0), stop=(st == NST - 1))
                    nc.vector.tensor_copy(wt_sb[:, dt, :], pw)
                # outT = Wv^T wT  [d-part, q], plus squared copy for RMS
                ot_sb = otp.tile([128, NDT, QB], F16, tag="ot", name="ot")
                sq_sb = sqp.tile([128, NDT, QB], F16, tag="sq", name="sq")
                for ot in range(NDT):
                    po = pswo.tile([128, QB], F32, tag="o", name="po", bufs=2)
                    for din in range(NDT):
                        nc.tensor.matmul(
                            po,
                            lhsT=wv_sb[:, din, ot * 128:(ot + 1) * 128],
                            rhs=wt_sb[:, din, :],
                            start=(din == 0), stop=(din == NDT - 1))
                    nc.vector.tensor_copy(ot_sb[:, ot, :], po)
                    nc.sync.dma_start(out=out_v[ot][:, qs:qs + QB],
                                      in_=ot_sb[:, ot, :])
                    nc.vector.tensor_mul(sq_sb[:, ot, :], ot_sb[:, ot, :],
                                         ot_sb[:, ot, :])
                pq = psdb.tile([128, QB], F32, tag="den", name="pq", bufs=1)
                for k in range(4):
                    for g in range(2):
                        ot = g * 4 + k
                        nc.tensor.matmul(
                            pq[32 * g:32 * g + 1, :], lhsT=ones_col,
                            rhs=sq_sb[:, ot, :],
                            start=(k == 0), stop=(k == 3),
                            tile_position=(0, 32 * g))
                q2c = work.tile([1, QB], F32, tag="q2c", name="q2c", bufs=1)
                nc.vector.tensor_copy(q2c, pq[0:1, :])
                q2s = work.tile([1, QB], F32, tag="q2s", name="q2s", bufs=1)
                nc.vector.tensor_add(q2s, q2c, pq[32:33, :])
                # rr = 1/sqrt(mean+eps) in one activation; (1-lambda_init)
                # folds into the broadcast's stationary row
                rms = work.tile([1, QB], F32, tag="rms", name="rms", bufs=1)
                nc.scalar.activation(rms, q2s, AF.Sqrt, scale=1.0 / D,
                                     bias=eps1)
                rr = work.tile([1, QB], F32, tag="rr", name="rr", bufs=1)
                nc.vector.reciprocal_approx_fast(rr, rms)
                nc.sync.dma_start(out=rrow.ap()[:, qs:qs + QB], in_=rr)

        pres_cm.__exit__(None, None, None)
        singles_cm.__exit__(None, None, None)

    nc.finalize()
    return nc


def get_nc():
    if "nc" not in _CACHE:
        _CACHE["nc"] = _build_nc()
    return _CACHE["nc"]


def make_in_maps(x, w_q12, w_k12, w_v, lambda_q1, lambda_k1, lambda_q2,
                 lambda_k2):
    wq = np.asarray(w_q12, np.float32)
    wk = np.asarray(w_k12, np.float32)
    m1 = wq[:, :D] @ wk[:, :D].T
    m2 = wq[:, D:] @ wk[:, D:].T
    mm_ = np.ascontiguousarray(
        np.concatenate([m1, m2], axis=1)).astype(np.float16)
    wv_ = np.asarray(w_v, np.float32).astype(np.float16)
    lam1 = np.exp(np.sum(np.asarray(lambda_q1, np.float64) *
                         np.asarray(lambda_k1, np.float64)))
    lam2 = np.exp(np.sum(np.asarray(lambda_q2, np.float64) *
                         np.asarray(lambda_k2, np.float64)))
    lamv = np.array([[lam1 - lam2 + LAMBDA_INIT]], dtype=np.float32)
    in_maps = []
    per_b = {}
    for b in range(B):
        xb = np.asarray(x[b], np.float32)
        xT_ = np.ascontiguousarray(xb.T).astype(np.float16)
        xsd_ = xb.astype(np.float16)
        per_b[b] = (xT_, xsd_)
    for c in range(NCORES):
        b, h = divmod(c, 2)
        xT_, xsd_ = per_b[b]
        if h:
            # roll the sequence so this core's q rows are columns [0, QH);
            # softmax and the wT s-contraction are permutation-invariant
            xT_ = np.ascontiguousarray(
                np.concatenate([xT_[:, QH:], xT_[:, :QH]], axis=1))
            xsd_ = np.ascontiguousarray(
                np.concatenate([xsd_[QH:], xsd_[:QH]], axis=0))
        in_maps.append({"xT": xT_, "xsd": xsd_, "mm": mm_,
                        "wv": wv_, "lamv": lamv})
    return in_maps


def kernel(x, w_q12, w_k12, w_v, lambda_q1, lambda_k1, lambda_q2, lambda_k2,
           **run_kwargs):
    nc = get_nc()
    in_maps = make_in_maps(x, w_q12, w_k12, w_v, lambda_q1, lambda_k1,
                           lambda_q2, lambda_k2)
    res = run_bass_kernel_spmd(nc, in_maps, list(range(NCORES)), **run_kwargs)
    _CACHE["last_result"] = res
    out = np.empty((B, S, D), dtype=np.float32)
    for c in range(NCORES):
        b, h = divmod(c, 2)
        rscale = (res.results[c]["rrow"][0].astype(np.float32) *
                  (1.0 - LAMBDA_INIT))
        out[b, h * QH:(h + 1) * QH, :] = (
            res.results[c]["out"].T.astype(np.float32) * rscale[:, None])
    return out
